# revision 16
# baseline (speedup 1.0000x reference)
"""GPT2 attention + adapter kernel for 8 Trainium2 NeuronCores.

Sharding: core r owns heads {2r, 2r+1} for BOTH batch elements (tensor
parallel over the 16 heads; both batches processed on every core).  All
matmul operands are bf16 (fp32 PSUM accumulation).

On-chip layout is transposed (feature dim on partitions, sequence free):
  qT/kT[b]  [2*64, S]  from  W^T @ x^T   (x^T supplied by host, bf16)
  scoresT pair psum [128, 1024] = [headA | headB], per batch
  P^T = exp(scores/8); causal: diagonal blocks only compute cols >= 128d,
        gpsimd affine_select masks just the triangular 128-col strip
  attn^T accum [65, q] = [v | 1].T @ P^T   (ones column -> softmax denom)
  after per-head normalization + gating, transposed attention slices are
  exchanged with an 8-core AllToAll (2 rounds of 1024 queries); each core
  receives all 16 heads' dims for its 128-query strips of both batches, so
  c_proj (full weight, loaded on every core) needs no cross-core reduction.

Issue order is software-pipelined: normalize(qc-1) and c_proj of completed
exchange rounds are injected into qc's score/PV stream so the PE never
drains.  Output rows on core r: [round t | batch b | 128 q] where the full
q index is t*1024 + r*128 + q'.
"""

import sys

for _p in ("/opt/trn_rl_repo",):
    if _p not in sys.path:
        sys.path.insert(0, _p)

import numpy as np

# ---------------------------------------------------------------- constants
B, S, A, D, H, HD = 2, 2048, 64, 1024, 16, 64
N_CORES = 8
SA = S + A       # 2112
SCALE = 1.0 / 8.0
P = 128
NQC = S // 512   # 4 query chunks of 512
NST = S // P     # 16 s-tiles of 128
VW = 2 * 65      # v_sb row width per s-tile (2 heads x (64 dims + ones col))

_STATE: dict = {}


def _build_nc(reps=1, collective=True):
    import concourse.bacc as bacc
    import concourse.mybir as mybir
    import concourse.tile as tile
    from concourse.alu_op_type import AluOpType

    f32 = mybir.dt.float32
    bf16 = mybir.dt.bfloat16
    AF = mybir.ActivationFunctionType

    nc = bacc.Bacc("TRN2", target_bir_lowering=False, debug=False,
                   num_devices=N_CORES)

    xa_t = nc.dram_tensor("xa_t", [16, P, SA], bf16, kind="ExternalInput").ap()
    w_qkv = nc.dram_tensor("w_qkv", [8, P, 384], bf16, kind="ExternalInput").ap()
    b_qkv = nc.dram_tensor("b_qkv", [P, 4], f32, kind="ExternalInput").ap()
    w_proj = nc.dram_tensor("w_proj", [8, P, D], bf16, kind="ExternalInput").ap()
    # packed constants: cstb[128,640] bf16 = [ones(64) | sden 8x8 | sbc 8x64],
    # cstf[64,5] f32 = [gscale col | bvef 4 cols]
    cstb_d = nc.dram_tensor("cstb", [P, 640], bf16, kind="ExternalInput").ap()
    cstf_d = nc.dram_tensor("cstf", [64, 5], f32, kind="ExternalInput").ap()
    out_ext = nc.dram_tensor("out", [512, D], f32, kind="ExternalOutput").ap()
    # dummy input whose shape encodes `reps` so each variant gets a distinct
    # HLO signature (the neuron compile cache ignores the embedded BIR)
    nc.dram_tensor("repsig", [reps, 1], f32, kind="ExternalInput")

    with tile.TileContext(nc) as tc, \
            nc.allow_low_precision(reason="bf16 matmul path"):
        with (
            tc.tile_pool(name="persist", bufs=1) as pp,
            tc.tile_pool(name="dram", bufs=1, space="DRAM") as dp,
        ):
            qT2 = [pp.tile([P, S], bf16, tag=f"qT{i}", name=f"qT{i}") for i in range(2)]
            kT2 = [pp.tile([P, S], bf16, tag=f"kT{i}", name=f"kT{i}") for i in range(2)]
            kaT2 = [pp.tile([P, A], bf16, tag=f"kaT{i}", name=f"kaT{i}") for i in range(2)]
            v_sb = [pp.tile([P, NST * VW], bf16, tag=f"v{i}", name=f"v{i}") for i in range(2)]
            va_sb = [pp.tile([64, VW], bf16, tag=f"va{i}", name=f"va{i}") for i in range(2)]
            # va2[b]: head 0 at parts 0:64 cols 0:65; head 1 at parts 64:128
            # cols 65:130 (so its pva matmul can use array rows 64:128)
            va2 = [pp.tile([P, 130], bf16, tag=f"va2_{i}", name=f"va2_{i}") for i in range(2)]
            # attn_h[j], j = b*2+h: normalized gated attention, transposed
            attn_h = [pp.tile([64, S], bf16, tag=f"at{j}", name=f"at{j}") for j in range(4)]
            # ax[b]: received dims for my q-strip of the current round
            ax = [pp.tile([P, 1024], bf16, tag=f"ax{i}", name=f"ax{i}") for i in range(2)]
            wproj_sb = [pp.tile([P, D], bf16, tag=f"wp{i}", name=f"wp{i}") for i in range(8)]
            cstb_sb = pp.tile([P, 640], bf16, tag="cstb", name="cstb")
            cstf_sb = pp.tile([64, 5], f32, tag="cstf", name="cstf")
            gsc_sb = cstf_sb[0:8, 0:1]
            bvef_sb = [cstf_sb[:, 1 + j:2 + j] for j in range(4)]
            sden_sb = [cstb_sb[0:65, 64 + 8 * j:64 + 8 * (j + 1)] for j in range(8)]
            sbc_sb = [cstb_sb[0:8, 128 + 64 * j:128 + 64 * (j + 1)] for j in range(8)]

            a2a_in = dp.tile([2, 8, 256, P], bf16, tag="a2ain", name="a2ain")
            a2a_out = dp.tile([2, 8, 256, P], bf16, tag="a2aout", name="a2aout")

            for _rep in range(reps):
                # ---------------- stage 1: projections ----------------
                with (
                    tc.tile_pool(name="s1in", bufs=1) as s1p,
                    tc.tile_pool(name="s1ps", bufs=1, space="PSUM") as ps1,
                    tc.tile_pool(name="s1b", bufs=1) as s1b,
                ):
                    w_sb = [s1p.tile([P, 384], bf16, tag=f"w{k}", name=f"w{k}")
                            for k in range(8)]
                    xa_sb = [s1p.tile([P, SA], bf16, tag=f"xa{k}", name=f"xa{k}")
                             for k in range(16)]
                    bias_t = s1b.tile([P, 4], f32, tag="bia", name="bia")
                    bias_sb = [bias_t[:, m:m + 1] for m in range(4)]
                    # interleave so chunk kc's operands arrive together;
                    # batch-0 xa first (m=0 needs it), batch-1 trails
                    for k in range(8):
                        nc.sync.dma_start(out=w_sb[k][:], in_=w_qkv[k])
                        nc.sync.dma_start(out=xa_sb[k][:], in_=xa_t[k])
                    nc.sync.dma_start(out=bias_t[:], in_=b_qkv[:])
                    for k in range(8, 16):
                        nc.sync.dma_start(out=xa_sb[k][:], in_=xa_t[k])
                    nc.sync.dma_start(out=cstb_sb[:], in_=cstb_d[:])
                    nc.sync.dma_start(out=cstf_sb[:], in_=cstf_d[:])
                    for i in range(8):
                        nc.sync.dma_start(out=wproj_sb[i][:], in_=w_proj[i])

                    for b in range(2):
                        v_ones = v_sb[b].rearrange("p (t c) -> p t c", c=65)[:, :, 64:65]
                        nc.vector.tensor_copy(
                            v_ones, cstb_sb[:, 0:2 * NST]
                            .rearrange("p (t c) -> p t c", c=1))
                        va_ones = va_sb[b].rearrange("p (t c) -> p t c", c=65)[:, :, 64:65]
                        nc.vector.tensor_copy(
                            va_ones, cstb_sb[0:64, 0:2]
                            .rearrange("p (t c) -> p t c", c=1))

                    # q/k transposed: psum[cols128, s512] over 8 K-chunks
                    # m -> (qk = m//2, b = m%2)
                    for m in range(4):
                        is_k = m >= 2
                        b = m % 2
                        wc = slice(P, 2 * P) if is_k else slice(0, P)
                        psl = [ps1.tile([P, 512], f32, tag=f"s1_{n}", name=f"s1_{n}")
                               for n in range(4)]
                        psla = (ps1.tile([P, A], f32, tag="s1_a", name="s1_a")
                                if is_k else None)
                        for kc in range(8):
                            lhs = w_sb[kc][:, wc]
                            for n in range(4):
                                nc.tensor.matmul(
                                    psl[n][:], lhs,
                                    xa_sb[b * 8 + kc][:, n * 512:(n + 1) * 512],
                                    start=(kc == 0), stop=(kc == 7))
                            if is_k:
                                nc.tensor.matmul(
                                    psla[:], lhs, xa_sb[b * 8 + kc][:, S:SA],
                                    start=(kc == 0), stop=(kc == 7))
                        tgt = kT2[b] if is_k else qT2[b]
                        for n in range(4):
                            nc.scalar.activation(
                                tgt[:, n * 512:(n + 1) * 512], psl[n][:],
                                AF.Identity, bias=bias_sb[m][:])
                        if is_k:
                            nc.scalar.activation(
                                kaT2[b][:], psla[:],
                                AF.Identity, bias=bias_sb[m][:])

                    # v natural: psum[s128, vcols128] over 8 K-chunks
                    for b in range(2):
                        for st in range(NST):
                            psv = ps1.tile([P, P], f32, tag="s1_v", name="s1_v",
                                           bufs=2)
                            for kc in range(8):
                                nc.tensor.matmul(
                                    psv[:],
                                    xa_sb[b * 8 + kc][:, st * P:(st + 1) * P],
                                    w_sb[kc][:, 256:384],
                                    start=(kc == 0), stop=(kc == 7))
                            vdst = v_sb[b].rearrange("p (t c) -> p t c", c=65)[
                                :, st * 2:(st + 1) * 2, 0:64]
                            nc.vector.tensor_copy(
                                vdst, psv.rearrange("p (h c) -> p h c", c=64))
                        psva = ps1.tile([64, P], f32, tag="s1_va", name="s1_va")
                        for kc in range(8):
                            nc.tensor.matmul(
                                psva[:], xa_sb[b * 8 + kc][:, S:SA],
                                w_sb[kc][:, 256:384],
                                start=(kc == 0), stop=(kc == 7))
                        vadst = va_sb[b].rearrange("p (h c) -> p h c", c=65)[:, :, 0:64]
                        nc.vector.tensor_copy(
                            vadst, psva.rearrange("p (h c) -> p h c", c=64))
                        nc.vector.tensor_copy(va2[b][0:64, 0:65],
                                              va_sb[b][0:64, 0:65])
                        nc.sync.dma_start(out=va2[b][64:128, 65:130],
                                          in_=va_sb[b][0:64, 65:130])

                # ---------------- stage 2: attention ----------------
                with (
                    tc.tile_pool(name="scps", bufs=2, space="PSUM") as scps,
                    tc.tile_pool(name="pvps", bufs=2, space="PSUM") as pvps,
                    tc.tile_pool(name="shps", bufs=2, space="PSUM") as shps,
                    tc.tile_pool(name="ptp", bufs=4) as ptp,
                    tc.tile_pool(name="osb", bufs=18) as osb,
                    tc.tile_pool(name="smal", bufs=4) as smal,
                    tc.tile_pool(name="ysbp", bufs=2) as ysbp,
                ):
                    o_hist = {}    # qc -> (o_main[4], o_adpt[4]) by j=b*2+h
                    rec_hist = {}  # qc -> rec bf16 tile

                    def adapter_attn(qc, b):
                        qs = slice(qc * 512, (qc + 1) * 512)
                        ssa = scps.tile([P, 1024], f32, tag="sc", name="sca")
                        nc.tensor.matmul(
                            ssa[0:64, 0:512], kaT2[b][0:64, :],
                            qT2[b][0:64, qs],
                            start=True, stop=True, tile_position=(0, 0))
                        nc.tensor.matmul(
                            ssa[64:128, 512:1024], kaT2[b][64:128, :],
                            qT2[b][64:128, qs],
                            start=True, stop=True, tile_position=(64, 64))
                        pta = ptp.tile([P, 1024], bf16, tag="pt", name="pta")
                        nc.scalar.activation(pta[0:64, 0:512], ssa[0:64, 0:512],
                                             AF.Exp, bias=0.0, scale=SCALE)
                        nc.scalar.activation(pta[64:128, 512:1024],
                                             ssa[64:128, 512:1024],
                                             AF.Exp, bias=0.0, scale=SCALE)
                        pvaA = pvps.tile([65, 512], f32, tag="pv", name="pva")
                        pvaB = pvps.tile([65, 512], f32, tag="pv", name="pva")
                        nc.tensor.matmul(
                            pvaA[:], va2[b][0:64, 0:65], pta[0:64, 0:512],
                            start=True, stop=True, tile_position=(0, 0))
                        nc.tensor.matmul(
                            pvaB[:], va2[b][64:128, 65:130], pta[64:128, 512:1024],
                            start=True, stop=True, tile_position=(64, 0))
                        oA = osb.tile([65, 512], bf16, tag="om", name="oa")
                        oB = osb.tile([65, 512], bf16, tag="om", name="oa")
                        nc.vector.tensor_copy(oA[:], pvaA[:])
                        nc.vector.tensor_copy(oB[:], pvaB[:])
                        return oA, oB

                    def gather_recip(qc):
                        """Denominators of qc -> rec bf16 [8,512]: rows 0:4
                        main heads (x1), rows 4:8 adapter (x tanh gate)."""
                        o_main, o_adpt = o_hist[qc]
                        dps_t = shps.tile([P, 512], f32, tag="sh", name="dps")
                        dps = dps_t[0:8, :]
                        tiles = list(o_main) + list(o_adpt)
                        for j in range(8):
                            nc.tensor.matmul(dps, sden_sb[j], tiles[j][:],
                                             start=(j == 0), stop=(j == 7))
                        rec_f = smal.tile([8, 512], f32, tag="rec", name="rec")
                        nc.vector.reciprocal(rec_f[:], dps)
                        rec = smal.tile([8, 512], bf16, tag="recb", name="recb")
                        nc.vector.tensor_scalar_mul(rec[:], rec_f[:], gsc_sb)
                        rec_hist[qc] = rec

                    def normalize_head(qc, j):
                        """Broadcast 1/den + combine for slot j=b*2+h of qc."""
                        qs = slice(qc * 512, (qc + 1) * 512)
                        o_main, o_adpt = o_hist[qc]
                        rec = rec_hist[qc]
                        rbm_t = shps.tile([P, 512], f32, tag="sh", name="rbm")
                        rbm = rbm_t[0:64, :]
                        nc.tensor.matmul(rbm, sbc_sb[j], rec[:],
                                         start=True, stop=True)
                        rba_t = shps.tile([P, 512], f32, tag="sh", name="rba")
                        rba = rba_t[0:64, :]
                        nc.tensor.matmul(rba, sbc_sb[4 + j], rec[:],
                                         start=True, stop=True)
                        t1 = smal.tile([64, 512], bf16, tag="t1", name="t1")
                        t2 = smal.tile([64, 512], bf16, tag="t2", name="t2")
                        nc.vector.tensor_tensor(t1[:], o_main[j][0:64, :],
                                                rbm, op=AluOpType.mult)
                        nc.vector.tensor_tensor(t2[:], o_adpt[j][0:64, :],
                                                rba, op=AluOpType.mult)
                        nc.vector.scalar_tensor_tensor(
                            attn_h[j][:, qs], t2[:], bvef_sb[j], t1[:],
                            op0=AluOpType.add, op1=AluOpType.add)

                    def stage_a2a(t):
                        """Exchange round t (queries t*1024 .. t*1024+1024)."""
                        for j in range(4):
                            nc.sync.dma_start(
                                out=a2a_in[t, :, j * 64:(j + 1) * 64, :]
                                    .rearrange("i p c -> p i c"),
                                in_=attn_h[j][:, t * 1024:(t + 1) * 1024]
                                    .rearrange("p (i c) -> p i c", i=8))
                        if collective:
                            nc.gpsimd.collective_compute(
                                "AllToAll", AluOpType.bypass,
                                replica_groups=[[0, 1, 2, 3, 4, 5, 6, 7]],
                                ins=[a2a_in[t]], outs=[a2a_out[t]])
                        else:
                            nc.gpsimd.dma_start(out=a2a_out[t], in_=a2a_in[t])
                        for b in range(2):
                            nc.sync.dma_start(
                                out=ax[b].rearrange("p (i c) -> p i c", i=8),
                                in_=a2a_out[t][:, b * P:(b + 1) * P, :]
                                    .rearrange("i p c -> p i c"))

                    def cproj(t, b):
                        """Final projection: round t, batch b -> out rows
                        t*256 + b*128 + [0:128]."""
                        ysb = ysbp.tile([P, 1024], f32, tag="y", name="y")
                        for dc in range(2):
                            psy = shps.tile([P, 512], f32, tag="sh", name="psy")
                            for i in range(8):
                                nc.tensor.matmul(
                                    psy[:],
                                    ax[b][:, i * P:(i + 1) * P],
                                    wproj_sb[i][:, dc * 512:(dc + 1) * 512],
                                    start=(i == 0), stop=(i == 7))
                            nc.vector.tensor_copy(
                                ysb[:, dc * 512:(dc + 1) * 512], psy[:])
                        nc.sync.dma_start(
                            out=out_ext[t * 256 + b * P:t * 256 + (b + 1) * P, :],
                            in_=ysb[:])

                    def score_pv_stream(qc, b, inject):
                        """Score->exp->mask->PV pipeline for batch b's head
                        pair.  inject: block-idx -> list of thunks issued
                        after that block's PV matmuls."""
                        nkb = 4 * qc + 4
                        DEPTH = 2
                        pvA = pvps.tile([65, 512], f32, tag="pv", name="pv")
                        pvB = pvps.tile([65, 512], f32, tag="pv", name="pv")
                        ss_l = [None] * nkb
                        pt_l = [None] * nkb

                        def issue_ss(kb):
                            d = kb - 4 * qc  # >=0 on diagonal blocks
                            c0 = max(d, 0) * P
                            ss = scps.tile([P, 1024], f32, tag="sc", name="sc")
                            kslc = slice(kb * P, (kb + 1) * P)
                            nc.tensor.matmul(
                                ss[:, c0:512], kT2[b][0:64, kslc],
                                qT2[b][0:64, qc * 512 + c0:(qc + 1) * 512],
                                start=True, stop=True, tile_position=(0, 0))
                            nc.tensor.matmul(
                                ss[:, 512 + c0:1024], kT2[b][64:128, kslc],
                                qT2[b][64:128, qc * 512 + c0:(qc + 1) * 512],
                                start=True, stop=True, tile_position=(64, 0))
                            ss_l[kb] = ss

                        def issue_exp(kb):
                            d = kb - 4 * qc
                            c0 = max(d, 0) * P
                            ss = ss_l[kb]
                            pt = ptp.tile([P, 1024], bf16, tag="pt", name="pt")
                            nc.scalar.activation(pt[:, c0:512], ss[:, c0:512],
                                                 AF.Exp, bias=0.0, scale=SCALE)
                            nc.scalar.activation(pt[:, 512 + c0:1024],
                                                 ss[:, 512 + c0:1024],
                                                 AF.Exp, bias=0.0, scale=SCALE)
                            if d >= 0:  # mask the triangular strip only
                                for half in range(2):
                                    strip = pt[:, half * 512 + c0:
                                               half * 512 + c0 + P]
                                    nc.gpsimd.affine_select(
                                        out=strip, in_=strip,
                                        compare_op=AluOpType.is_ge,
                                        fill=0.0, base=0,
                                        pattern=[[1, P]],
                                        channel_multiplier=-1)
                            ss_l[kb] = None
                            pt_l[kb] = pt

                        def issue_pv(kb):
                            d = kb - 4 * qc
                            c0 = max(d, 0) * P
                            pt = pt_l[kb]
                            st = kb
                            nc.tensor.matmul(
                                pvA[:, c0:512],
                                v_sb[b][:, st * VW: st * VW + 65],
                                pt[0:P, c0:512],
                                start=(kb == 0), stop=(kb == nkb - 1))
                            nc.tensor.matmul(
                                pvB[:, c0:512],
                                v_sb[b][:, st * VW + 65: st * VW + 130],
                                pt[0:P, 512 + c0:1024],
                                start=(kb == 0), stop=(kb == nkb - 1))
                            pt_l[kb] = None

                        for kb in range(min(DEPTH, nkb)):
                            issue_ss(kb)
                            issue_exp(kb)
                        for kb in range(nkb):
                            if kb + DEPTH < nkb:
                                issue_ss(kb + DEPTH)
                                issue_exp(kb + DEPTH)
                            issue_pv(kb)
                            for thunk in inject.get(kb, ()):
                                thunk()
                        oA = osb.tile([65, 512], bf16, tag="om", name="om")
                        oB = osb.tile([65, 512], bf16, tag="om", name="om")
                        nc.vector.tensor_copy(oA[:], pvA[:])
                        nc.vector.tensor_copy(oB[:], pvB[:])
                        return oA, oB

                    for qc in range(NQC):
                        o_main = [None] * 4
                        o_adpt = [None] * 4

                        # batch-0 stream; inject gather(qc-1) + cproj(round 0)
                        inj0 = {}
                        if qc >= 1:
                            inj0.setdefault(1, []).append(
                                lambda q=qc - 1: gather_recip(q))
                        if qc == 3:
                            inj0.setdefault(3, []).append(lambda: cproj(0, 0))
                            inj0.setdefault(5, []).append(lambda: cproj(0, 1))
                        o_main[0], o_main[1] = score_pv_stream(qc, 0, inj0)
                        o_adpt[0], o_adpt[1] = adapter_attn(qc, 0)

                        # batch-1 stream; inject normalize(qc-1) spread out
                        inj1 = {}
                        if qc >= 1:
                            for j in range(4):
                                inj1.setdefault(min(2 * j, 4 * qc + 2), []).append(
                                    lambda q=qc - 1, jj=j: normalize_head(q, jj))
                        o_main[2], o_main[3] = score_pv_stream(qc, 1, inj1)
                        o_adpt[2], o_adpt[3] = adapter_attn(qc, 1)

                        o_hist[qc] = (o_main, o_adpt)
                        if qc == 2:
                            # rows 0:1024 fully normalized (during qc=2)
                            stage_a2a(0)

                    # tail: normalize qc=3, exchange round 1, final cprojs
                    gather_recip(3)
                    for j in range(4):
                        normalize_head(3, j)
                    stage_a2a(1)
                    cproj(1, 0)
                    cproj(1, 1)

    nc.compile()
    return nc


def _make_in_maps(hidden_states, adapter, c_attn_w, c_attn_b, c_proj_w, gate):
    import ml_dtypes
    bf16 = ml_dtypes.bfloat16

    hidden_states = np.asarray(hidden_states, np.float32)
    adapter = np.asarray(adapter, np.float32)
    c_attn_w = np.asarray(c_attn_w, np.float32)
    c_attn_b = np.asarray(c_attn_b, np.float32)
    c_proj_w = np.asarray(c_proj_w, np.float32)
    gate = np.asarray(gate, np.float32)

    # packed bf16 constants: [ones(64) | sden 8x8 | sbc 8x64]
    cstb = np.zeros((P, 640), np.float32)
    cstb[:, 0:64] = 1.0
    for j in range(8):
        cstb[64, 64 + 8 * j + j] = 1.0          # sden[j]: row 64, col j
        cstb[j, 128 + 64 * j:128 + 64 * (j + 1)] = 1.0  # sbc[j]: row j ones
    cstb = cstb.astype(bf16)
    w_proj = np.ascontiguousarray(c_proj_w.reshape(8, P, D)).astype(bf16)

    # xa_t: both batches, transposed, [16, 128, 2112] (batch-major chunks)
    xa_t = np.empty((2, D, SA), np.float32)
    for b in range(2):
        xa = np.concatenate([hidden_states[b], adapter[b]], axis=0)
        xa_t[b] = xa.T
    xa_t = np.ascontiguousarray(xa_t.reshape(16, P, SA)).astype(bf16)

    in_maps = []
    for r in range(N_CORES):
        cs = slice(r * P, (r + 1) * P)  # my 2 heads' feature columns
        w_qkv = np.concatenate(
            [c_attn_w[:, cs], c_attn_w[:, D:][:, cs], c_attn_w[:, 2 * D:][:, cs]],
            axis=1).reshape(8, P, 384)
        b_q = c_attn_b[cs]
        b_k = c_attn_b[D:][cs]
        b_v = c_attn_b[2 * D:][cs]
        b_qkv = np.stack([b_q, b_q, b_k, b_k], axis=1)     # [128, 4]
        tg = np.tanh(gate[0, 2 * r:2 * r + 2, 0, 0])       # my 2 heads
        bv_e = (b_v.reshape(2, HD) * (1.0 + tg)[:, None])
        bv_e = np.tile(bv_e, (2, 1))                       # [4, 64], j = b*2+h
        cstf = np.zeros((64, 5), np.float32)
        cstf[0:4, 0] = 1.0
        cstf[4:8, 0] = np.tile(tg, 2)
        cstf[:, 1:5] = bv_e.T
        in_maps.append({
            "xa_t": xa_t,
            "w_qkv": np.ascontiguousarray(w_qkv).astype(bf16),
            "b_qkv": np.ascontiguousarray(b_qkv).astype(np.float32),
            "w_proj": w_proj,
            "cstb": cstb,
            "cstf": cstf,
        })
    return in_maps


def _get_runner(reps=1):
    """Build + compile once; return f(in_maps) -> list[dict] (per-core)."""
    key = ("run", reps)
    if key in _STATE:
        return _STATE[key]

    import jax
    import jax.numpy as jnp  # noqa: F401
    from jax.experimental.shard_map import shard_map
    from jax.sharding import Mesh, PartitionSpec

    import concourse.mybir as mybir
    from concourse import bass2jax

    nc = _build_nc(reps)
    bass2jax.install_neuronx_cc_hook()

    partition_name = (nc.partition_id_tensor.name
                      if nc.partition_id_tensor else None)
    in_names, out_names, out_avals, zero_outs = [], [], [], []
    for alloc in nc.m.functions[0].allocations:
        if not isinstance(alloc, mybir.MemoryLocationSet):
            continue
        name = alloc.memorylocations[0].name
        if alloc.kind == "ExternalInput":
            if name != partition_name:
                in_names.append(name)
        elif alloc.kind == "ExternalOutput":
            shape = tuple(alloc.tensor_shape)
            dtype = mybir.dt.np(alloc.dtype)
            out_names.append(name)
            out_avals.append(jax.core.ShapedArray(shape, dtype))
            zero_outs.append(np.zeros(shape, dtype))
    in_shapes = {}
    for alloc in nc.m.functions[0].allocations:
        if isinstance(alloc, mybir.MemoryLocationSet) and alloc.kind == "ExternalInput":
            in_shapes[alloc.memorylocations[0].name] = (
                tuple(alloc.tensor_shape), mybir.dt.np(alloc.dtype))
    n_params = len(in_names)
    n_outs = len(out_avals)
    all_in_names = list(in_names) + list(out_names)
    if partition_name is not None:
        all_in_names.append(partition_name)
    donate = tuple(range(n_params, n_params + n_outs))

    def _body(*args):
        operands = list(args)
        if partition_name is not None:
            operands.append(bass2jax.partition_id_tensor())
        outs = bass2jax._bass_exec_p.bind(
            *operands,
            out_avals=tuple(out_avals),
            in_names=tuple(all_in_names),
            out_names=tuple(out_names),
            lowering_input_output_aliases=(),
            sim_require_finite=True,
            sim_require_nnan=True,
            nc=nc,
        )
        return tuple(outs)

    devices = jax.devices()[:N_CORES]
    mesh = Mesh(np.asarray(devices), ("core",))
    in_specs = (PartitionSpec("core"),) * (n_params + n_outs)
    out_specs = (PartitionSpec("core"),) * n_outs
    sharded = jax.jit(
        shard_map(_body, mesh=mesh, in_specs=in_specs, out_specs=out_specs,
                  check_rep=False),
        donate_argnums=donate, keep_unused=True)

    def run(in_maps, as_np=True):
        def get(c, n):
            if n in in_maps[c]:
                return np.asarray(in_maps[c][n])
            shape, dt_ = in_shapes[n]
            return np.zeros(shape, dt_)
        concat_in = [
            np.concatenate([get(c, n) for c in range(N_CORES)], axis=0)
            for n in in_names
        ]
        concat_zeros = [
            np.zeros((N_CORES * z.shape[0], *z.shape[1:]), z.dtype)
            for z in zero_outs
        ]
        out_arrs = sharded(*concat_in, *concat_zeros)
        if not as_np:
            return out_arrs
        return [
            {n: np.asarray(out_arrs[i]).reshape(N_CORES, *out_avals[i].shape)[c]
             for i, n in enumerate(out_names)}
            for c in range(N_CORES)
        ]

    run.in_names = in_names
    run.in_shapes = in_shapes
    run.out_names = out_names
    run.zero_outs = zero_outs
    run.sharded = sharded
    _STATE[key] = run
    return run


def _assemble(results, c_proj_b):
    """Per-core [512, D] outputs -> full [B, S, D].
    Core r, round t, batch b: rows t*256+b*128+[0:128] are the full-model
    query rows t*1024 + r*128 + [0:128] of batch b."""
    out = np.empty((B, S, D), np.float32)
    for r in range(N_CORES):
        res = results[r]["out"]
        for t in range(2):
            for b in range(B):
                out[b, t * 1024 + r * P: t * 1024 + (r + 1) * P, :] = \
                    res[t * 256 + b * P: t * 256 + (b + 1) * P, :]
    out += np.asarray(c_proj_b, np.float32)
    return out


def kernel(hidden_states, adapter, c_attn_w, c_attn_b, c_proj_w, c_proj_b,
           gate):
    run = _get_runner()
    in_maps = _make_in_maps(hidden_states, adapter, c_attn_w, c_attn_b,
                            c_proj_w, gate)
    results = run(in_maps)
    return _assemble(results, c_proj_b)
